# revision 1
# baseline (speedup 1.0000x reference)
"""Causal self-attention (GQA + RoPE) Trainium2 kernel.

Full-input contract: kernel(**inputs) takes the unsharded tensors and returns
the full [B, T, C] output. Internally shards over 8 NeuronCores as
(batch b in {0,1}) x (kv-head group g in {0..3}); each core computes the
attention output of its 4 query heads (one kv head) for its batch and the
partial out-projection against its 512 rows of Wo. The host sums the 4 group
partials per batch.

Per-core dataflow (all tensor-engine matmuls in float32r, fp32 PSUM accum):
  phase A: QT[d,t] = Wq_g^T x^T, KT, VT streamed over C-chunks; RoPE applied
           with host-precomputed transposed cos/sin tables (score scale folded
           into Wq, rotate-half sign folded into the sin table); V transposed
           to [t,d] via PE.
  phase B (per 512-query block): S^T[k,q] tiles on PE (F restricted to the
           causally-valid columns on diagonal tiles), triangle mask add (DVE),
           exp (ACT, PSUM->SBUF) with GpSimd zero-fill of the masked column
           range, denominator via all-ones matmul broadcast into PSUM,
           attn@V accumulated per head, normalization as a tensor-tensor
           divide, then the out-projection rows of this query block.
"""

import sys

for _p in ("/opt/trn_rl_repo", "/root/.axon_site/_ro/trn_rl_repo"):
    if _p not in sys.path:
        sys.path.append(_p)

import numpy as np
from contextlib import ExitStack

import concourse.bass as bass
import concourse.bacc as bacc
import concourse.tile as tile
import concourse.mybir as mybir
from concourse.bass_utils import run_bass_kernel_spmd

F32 = mybir.dt.float32
F32R = mybir.dt.float32r

B, T, C = 2, 2048, 2048
N_HEADS, N_KV_HEADS, HD = 16, 4, 128
G = N_HEADS // N_KV_HEADS  # heads per group = 4
GW = G * HD  # 512, per-group Q width / Wo row count
N_CORES = 8
TC = 512  # q-block width
NTC = T // TC  # 4
NKT = T // HD  # 16 k-tiles of 128
NCC = C // 128  # 16 contraction chunks
MASK_NEG = -1.0e30

_prog_cache = {}


def _build_program():
    nc = bacc.Bacc(
        "TRN2",
        target_bir_lowering=False,
        debug=False,
        enable_asserts=False,
        num_devices=N_CORES,
    )

    xT = nc.dram_tensor("xT", [C, T], F32, kind="ExternalInput").ap()
    wq = nc.dram_tensor("wq", [C, GW], F32, kind="ExternalInput").ap()
    wk = nc.dram_tensor("wk", [C, HD], F32, kind="ExternalInput").ap()
    wv = nc.dram_tensor("wv", [C, HD], F32, kind="ExternalInput").ap()
    wo = nc.dram_tensor("wo", [GW, C], F32, kind="ExternalInput").ap()
    cos = nc.dram_tensor("cos", [HD, T], F32, kind="ExternalInput").ap()
    sin = nc.dram_tensor("sin", [HD, T], F32, kind="ExternalInput").ap()
    masks = nc.dram_tensor("masks", [128, 128], F32, kind="ExternalInput").ap()
    ident = nc.dram_tensor("ident", [128, 128], F32, kind="ExternalInput").ap()
    onesfull = nc.dram_tensor("onesfull", [128, 128], F32, kind="ExternalInput").ap()
    y = nc.dram_tensor("y", [T, C], F32, kind="ExternalOutput").ap()

    with tile.TileContext(nc) as tc, ExitStack() as ctx:
        big_pool = ctx.enter_context(tc.tile_pool(name="big", bufs=1))

        # big activations: QT [d, h*T + t], KT [d, t], V [t-part, kt*HD + d]
        qt_sb = big_pool.tile([128, G * T], F32R)
        kt_sb = big_pool.tile([128, T], F32R)
        v_sb = big_pool.tile([128, NKT * HD], F32R)

        # ---------------- phase A: projections + rope ----------------
        with ExitStack() as pa:
            wpool = pa.enter_context(tc.tile_pool(name="wpool", bufs=1))
            xin = pa.enter_context(tc.tile_pool(name="xin", bufs=10))
            rp = pa.enter_context(tc.tile_pool(name="rp", bufs=3))
            qt_ps_pool = pa.enter_context(tc.tile_pool(name="qtps", bufs=4, space="PSUM"))
            warm_pool = pa.enter_context(tc.tile_pool(name="warm", bufs=1, space="PSUM"))
            kv_ps_pool = pa.enter_context(tc.tile_pool(name="kvps", bufs=2, space="PSUM"))
            tp_ps_pool = pa.enter_context(tc.tile_pool(name="tpps", bufs=1, space="PSUM"))

            # weight-chunk DMAs are interleaved into the first t-chunk's
            # c-loop so the first matmuls start as soon as chunk 0 lands
            wq_sb = wpool.tile([128, NCC * GW], F32R)  # [c-chunk p, ci*512 + j]
            wk_sb = wpool.tile([128, NCC * HD], F32R)
            wv_sb = wpool.tile([128, NCC * HD], F32R)
            cos_sb = wpool.tile([HD, T], F32)
            sin_sb = wpool.tile([HD, T], F32)
            ident_sb = wpool.tile([128, 128], F32)

            def load_w_chunk(ci):
                nc.sync.dma_start(
                    wq_sb[:, ci * GW : (ci + 1) * GW],
                    wq[ci * 128 : (ci + 1) * 128, :].bitcast(F32R),
                )
                nc.sync.dma_start(
                    wk_sb[:, ci * HD : (ci + 1) * HD],
                    wk[ci * 128 : (ci + 1) * 128, :].bitcast(F32R),
                )
                nc.sync.dma_start(
                    wv_sb[:, ci * HD : (ci + 1) * HD],
                    wv[ci * 128 : (ci + 1) * 128, :].bitcast(F32R),
                )

            for tci in range(NTC):
                ts = slice(tci * TC, (tci + 1) * TC)
                qt_ps = [
                    qt_ps_pool.tile([128, TC], F32, tag="qtps", name=f"qtps{tci}_{j}")
                    for j in range(G)
                ]
                kt_ps = kv_ps_pool.tile([128, TC], F32, tag="kvps", name=f"ktps{tci}")
                vt_ps = kv_ps_pool.tile([128, TC], F32, tag="kvps", name=f"vtps{tci}")
                for ci in range(NCC):
                    if tci == 0:
                        load_w_chunk(ci)
                        if ci == 10:
                            nc.gpsimd.dma_start(cos_sb[:], cos[:])
                            nc.gpsimd.dma_start(sin_sb[:], sin[:])
                            nc.gpsimd.dma_start(ident_sb[:], ident[:])
                    x_sb = xin.tile([128, TC], F32R, tag="x", name=f"x{tci}_{ci}")
                    xq = nc.gpsimd if (tci == 0 and ci % 2 == 1) else nc.sync
                    xq.dma_start(
                        x_sb[:], xT[ci * 128 : (ci + 1) * 128, ts].bitcast(F32R)
                    )
                    st, sp = (ci == 0), (ci == NCC - 1)
                    for j in range(G):
                        nc.tensor.matmul(
                            qt_ps[j][:],
                            wq_sb[:, ci * GW + j * HD : ci * GW + (j + 1) * HD],
                            x_sb[:],
                            start=st,
                            stop=sp,
                        )
                    nc.tensor.matmul(
                        kt_ps[:],
                        wk_sb[:, ci * HD : (ci + 1) * HD],
                        x_sb[:],
                        start=st,
                        stop=sp,
                    )
                    nc.tensor.matmul(
                        vt_ps[:],
                        wv_sb[:, ci * HD : (ci + 1) * HD],
                        x_sb[:],
                        start=st,
                        stop=sp,
                    )
                    # HAM warm-keeper: trivial matmul with no DMA dependency
                    # beyond the already-needed weight chunk keeps the PE
                    # activity window from idling during DMA-limited spans
                    wtile = warm_pool.tile([8, 8], F32, tag="warm", name=f"wm{tci}_{ci}")
                    nc.tensor.matmul(
                        wtile[:],
                        wq_sb[:, ci * GW : ci * GW + 8],
                        wq_sb[:, ci * GW : ci * GW + 8],
                        start=True,
                        stop=True,
                    )

                # rope on Q heads: out = q*cos + swap(q)*sin_signed.
                # The single ACT copy is the only psum reader, so the
                # accumulator bank frees as early as possible.
                for j in range(G):
                    q_raw = rp.tile([128, TC], F32, tag="qraw", name=f"qraw{tci}_{j}")
                    nc.scalar.copy(q_raw[:], qt_ps[j][:])
                    t1 = rp.tile([128, TC], F32, tag="t1", name=f"t1_{tci}_{j}")
                    nc.vector.tensor_mul(t1[:], q_raw[:], cos_sb[:, ts])
                    qsw = rp.tile([128, TC], F32, tag="qsw", name=f"qsw{tci}_{j}")
                    nc.gpsimd.dma_start(qsw[0:64, :], q_raw[64:128, :])
                    nc.gpsimd.dma_start(qsw[64:128, :], q_raw[0:64, :])
                    t2 = rp.tile([128, TC], F32, tag="t2", name=f"t2_{tci}_{j}")
                    nc.vector.tensor_mul(t2[:], qsw[:], sin_sb[:, ts])
                    nc.vector.tensor_add(
                        qt_sb[:, j * T + tci * TC : j * T + (tci + 1) * TC], t1[:], t2[:]
                    )
                # rope on K
                k_raw = rp.tile([128, TC], F32, tag="qraw", name=f"kraw{tci}")
                nc.scalar.copy(k_raw[:], kt_ps[:])
                t1k = rp.tile([128, TC], F32, tag="t1", name=f"t1k{tci}")
                nc.vector.tensor_mul(t1k[:], k_raw[:], cos_sb[:, ts])
                ksw = rp.tile([128, TC], F32, tag="qsw", name=f"ksw{tci}")
                nc.gpsimd.dma_start(ksw[0:64, :], k_raw[64:128, :])
                nc.gpsimd.dma_start(ksw[64:128, :], k_raw[0:64, :])
                t2k = rp.tile([128, TC], F32, tag="t2", name=f"t2k{tci}")
                nc.vector.tensor_mul(t2k[:], ksw[:], sin_sb[:, ts])
                nc.vector.tensor_add(kt_sb[:, ts], t1k[:], t2k[:])

                for wj in range(3):
                    wtile = warm_pool.tile(
                        [8, 8], F32, tag="warm", name=f"wmr{tci}_{wj}"
                    )
                    nc.tensor.matmul(
                        wtile[:], wq_sb[:, 0:8], wq_sb[:, 0:8], start=True, stop=True
                    )

                # V: [d, t] psum -> sbuf, then PE-transpose to [t, d]
                vt_f = rp.tile([128, TC], F32, tag="vtf", name=f"vtf{tci}")
                nc.scalar.copy(vt_f[:], vt_ps[:])
                for s in range(TC // 128):
                    kt_i = tci * (TC // 128) + s
                    tp_ps = tp_ps_pool.tile([128, 128], F32, tag="tp", name=f"tp{kt_i}")
                    nc.tensor.transpose(
                        tp_ps[:], vt_f[:, s * 128 : (s + 1) * 128], ident_sb[:]
                    )
                    nc.scalar.copy(v_sb[:, kt_i * HD : (kt_i + 1) * HD], tp_ps[:])

        # -------- phase B: attention + out-projection per q-block --------
        with ExitStack() as pb:
            st_pool = pb.enter_context(tc.tile_pool(name="stps", bufs=4, space="PSUM"))
            ot_ps_pool = pb.enter_context(tc.tile_pool(name="otps", bufs=2, space="PSUM"))
            s_ps_pool = pb.enter_context(tc.tile_pool(name="sps", bufs=2, space="PSUM"))
            pt_pool = pb.enter_context(tc.tile_pool(name="pt", bufs=10))
            nrm_pool = pb.enter_context(tc.tile_pool(name="nrm", bufs=3))
            ot_sb_pool = pb.enter_context(tc.tile_pool(name="otsb", bufs=2))
            y_sb_pool = pb.enter_context(tc.tile_pool(name="ysb", bufs=3))
            bconst = pb.enter_context(tc.tile_pool(name="bconst", bufs=1))

            mask_sb = bconst.tile([128, 128], F32)
            nc.sync.dma_start(mask_sb[:], masks[:])
            onesfull_sb = bconst.tile([128, 128], F32R)
            nc.sync.dma_start(onesfull_sb[:], onesfull.bitcast(F32R))
            wo_sb = bconst.tile([128, G * C], F32R)  # [j in head-chunk, h*C + c]
            for h in range(G):
                nc.sync.dma_start(
                    wo_sb[:, h * C : (h + 1) * C],
                    wo[h * 128 : (h + 1) * 128, :].bitcast(F32R),
                )

            for qb in range(NTC):
                nkt = (qb + 1) * (TC // 128)
                # ot: [d, h*TC + q] for this q-block
                ot_qb = ot_sb_pool.tile([128, G * TC], F32R, tag="ot", name=f"ot{qb}")
                for hg in range(G // 2):  # head pairs to fit PSUM
                    ot_ps = [
                        ot_ps_pool.tile(
                            [128, TC], F32, tag="otps", name=f"otps{qb}_{hg}_{hh}"
                        )
                        for hh in range(2)
                    ]
                    sb_ps = [
                        s_ps_pool.tile(
                            [128, TC], F32, tag="sps", name=f"sps{qb}_{hg}_{hh}"
                        )
                        for hh in range(2)
                    ]
                    for kt in range(nkt):
                        dj = kt - 4 * qb
                        f0 = max(dj, 0) * 128  # first causally-valid column
                        st, sp = (kt == 0), (kt == nkt - 1)
                        pts = []
                        for hh in range(2):
                            h = 2 * hg + hh
                            s_t = st_pool.tile(
                                [128, TC], F32, tag="st", name=f"st{qb}_{kt}_{h}"
                            )
                            nc.tensor.matmul(
                                s_t[:, f0:TC],
                                kt_sb[:, kt * 128 : (kt + 1) * 128],
                                qt_sb[:, h * T + qb * TC + f0 : h * T + (qb + 1) * TC],
                                start=True,
                                stop=True,
                            )
                            if dj >= 0:
                                nc.vector.tensor_add(
                                    s_t[:, f0 : f0 + 128],
                                    s_t[:, f0 : f0 + 128],
                                    mask_sb[:],
                                )
                            pt = pt_pool.tile(
                                [128, TC], F32R, tag="pt", name=f"pt{qb}_{kt}_{h}"
                            )
                            if f0 > 0:
                                nc.vector.memset(pt[:, 0:f0].bitcast(F32), 0.0)
                            nc.scalar.activation(
                                pt[:, f0:TC],
                                s_t[:, f0:TC],
                                mybir.ActivationFunctionType.Exp,
                            )
                            pts.append(pt)
                        for hh in range(2):
                            nc.tensor.matmul(
                                sb_ps[hh][:], onesfull_sb[:], pts[hh][:], start=st, stop=sp
                            )
                            nc.tensor.matmul(
                                ot_ps[hh][:],
                                v_sb[:, kt * HD : (kt + 1) * HD],
                                pts[hh][:],
                                start=st,
                                stop=sp,
                            )
                    for hh in range(2):
                        h = 2 * hg + hh
                        r_f = nrm_pool.tile([128, TC], F32, tag="rf", name=f"rf{qb}_{h}")
                        nc.vector.reciprocal_approx_fast(r_f[:], sb_ps[hh][:])
                        nc.vector.tensor_mul(
                            ot_qb[:, h * TC : (h + 1) * TC],
                            ot_ps[hh][:],
                            r_f[:],
                        )

                # out-projection for this q-block
                for tl in range(TC // 128):
                    tsub = qb * (TC // 128) + tl
                    for cc in range(C // TC):
                        y_ps = st_pool.tile(
                            [128, TC], F32, tag="st", name=f"yps{tsub}_{cc}"
                        )
                        for h in range(G):
                            nc.tensor.matmul(
                                y_ps[:],
                                ot_qb[:, h * TC + tl * 128 : h * TC + (tl + 1) * 128],
                                wo_sb[:, h * C + cc * TC : h * C + (cc + 1) * TC],
                                start=(h == 0),
                                stop=(h == G - 1),
                            )
                        y_sb = y_sb_pool.tile(
                            [128, TC], F32, tag="ysb", name=f"ysb{tsub}_{cc}"
                        )
                        nc.vector.tensor_copy(y_sb[:], y_ps[:])
                        nc.sync.dma_start(
                            y[tsub * 128 : (tsub + 1) * 128, cc * TC : (cc + 1) * TC],
                            y_sb[:],
                        )

    nc.compile()
    return nc


def _rope_tables():
    theta = 1.0 / (10000.0 ** (np.arange(0, HD, 2, dtype=np.float32) / HD))
    freqs = np.arange(T, dtype=np.float32)[:, None] * theta[None, :]  # [T, 64]
    cos = np.concatenate([np.cos(freqs), np.cos(freqs)], axis=-1)  # [T, 128]
    sin = np.concatenate([np.sin(freqs), np.sin(freqs)], axis=-1)
    cosT = np.ascontiguousarray(cos.T).astype(np.float32)  # [128, T]
    sinT = np.ascontiguousarray(sin.T).astype(np.float32)
    sign = np.where(np.arange(HD) < HD // 2, np.float32(-1.0), np.float32(1.0))[:, None]
    sinT_signed = (sinT * sign).astype(np.float32)
    return cosT, sinT_signed


def _masks():
    p = np.arange(128)[:, None]
    f = np.arange(128)[None, :]
    return np.where(p <= f, 0.0, MASK_NEG).astype(np.float32)


def make_in_maps(x, Wq, Wk, Wv, Wo):
    x = np.asarray(x, dtype=np.float32)
    Wq = np.asarray(Wq, dtype=np.float32)
    Wk = np.asarray(Wk, dtype=np.float32)
    Wv = np.asarray(Wv, dtype=np.float32)
    Wo = np.asarray(Wo, dtype=np.float32)

    cosT, sinT = _rope_tables()
    masks = _masks()
    qscale = np.float32(1.0 / np.sqrt(HD))
    ident = np.eye(128, dtype=np.float32)
    onesfull = np.ones((128, 128), dtype=np.float32)

    in_maps = []
    for c in range(N_CORES):
        b, g = divmod(c, N_KV_HEADS)
        in_maps.append(
            {
                "xT": np.ascontiguousarray(x[b].T),
                "wq": np.ascontiguousarray(Wq[:, g * GW : (g + 1) * GW]) * qscale,
                "wk": np.ascontiguousarray(Wk[:, g * HD : (g + 1) * HD]),
                "wv": np.ascontiguousarray(Wv[:, g * HD : (g + 1) * HD]),
                "wo": np.ascontiguousarray(Wo[g * GW : (g + 1) * GW, :]),
                "cos": cosT,
                "sin": sinT,
                "masks": masks,
                "ident": ident,
                "onesfull": onesfull,
            }
        )
    return in_maps


def kernel(x, Wq, Wk, Wv, Wo):
    if "nc" not in _prog_cache:
        _prog_cache["nc"] = _build_program()
    nc = _prog_cache["nc"]

    in_maps = make_in_maps(x, Wq, Wk, Wv, Wo)
    res = run_bass_kernel_spmd(nc, in_maps, list(range(N_CORES)))
    _prog_cache["last_results"] = res

    out = np.zeros((B, T, C), dtype=np.float32)
    for c in range(N_CORES):
        b = c // N_KV_HEADS
        out[b] += res.results[c]["y"]
    return out



# revision 14
# speedup vs baseline: 1.1657x; 1.1657x over previous
"""Causal self-attention (GQA + RoPE) Trainium2 kernel.

Full-input contract: kernel(**inputs) takes the unsharded tensors and returns
the full [B, T, C] output. Internally shards over 8 NeuronCores as
(batch b in {0,1}) x (kv-head group g in {0..3}); each core computes the
attention output of its 4 query heads (one kv head) for its batch and the
partial out-projection against its 512 rows of Wo. The host sums the 4 group
partials per batch.

Per-core dataflow: projection (tci) and attention (qb=tci) segments are
interleaved so the PE never sits at a phase boundary; causality guarantees
attention block qb only needs K/V/Q through t-chunk qb. All matmul lhsT
operands are bf16 (1.0-cycle/row LDWEIGHTS), rhs operands bf16 or f32r.
Softmax denominators are accumulated as running sums on DVE/GpSimd (pacc)
with a single ones-matmul per (head, q-block) instead of one per k-tile.
S-matmuls are software-pipelined one k-tile ahead of the AV/accumulate ops
so the exp latency stays off the PE critical path.
"""

import sys

for _p in ("/opt/trn_rl_repo", "/root/.axon_site/_ro/trn_rl_repo"):
    if _p not in sys.path:
        sys.path.append(_p)

import numpy as np
import ml_dtypes
from contextlib import ExitStack

import concourse.bass as bass
import concourse.bacc as bacc
import concourse.tile as tile
import concourse.mybir as mybir
from concourse.bass_utils import run_bass_kernel_spmd

F32 = mybir.dt.float32
F32R = mybir.dt.float32r
BF16 = mybir.dt.bfloat16

B, T, C = 2, 2048, 2048
N_HEADS, N_KV_HEADS, HD = 16, 4, 128
G = N_HEADS // N_KV_HEADS  # heads per group = 4
GW = G * HD  # 512, per-group Q width / Wo row count
N_CORES = 8
TC = 512  # q-block width
NTC = T // TC  # 4
NKT = T // HD  # 16 k-tiles of 128
NCC = C // 128  # 16 contraction chunks
MASK_NEG = -1.0e30

_prog_cache = {}


def _build_program():
    nc = bacc.Bacc(
        "TRN2",
        target_bir_lowering=False,
        debug=False,
        enable_asserts=False,
        num_devices=N_CORES,
    )

    xT = nc.dram_tensor("xT", [C, T], BF16, kind="ExternalInput").ap()
    wq = nc.dram_tensor("wq", [C, GW], BF16, kind="ExternalInput").ap()
    wk = nc.dram_tensor("wk", [C, HD], BF16, kind="ExternalInput").ap()
    wv = nc.dram_tensor("wv", [C, HD], BF16, kind="ExternalInput").ap()
    wo = nc.dram_tensor("wo", [GW, C], BF16, kind="ExternalInput").ap()
    cos = nc.dram_tensor("cos", [HD, T], F32, kind="ExternalInput").ap()
    sin = nc.dram_tensor("sin", [HD, T], F32, kind="ExternalInput").ap()
    masks = nc.dram_tensor("masks", [128, 128], F32, kind="ExternalInput").ap()
    ident = nc.dram_tensor("ident", [128, 128], BF16, kind="ExternalInput").ap()
    onesfull = nc.dram_tensor("onesfull", [128, 128], F32, kind="ExternalInput").ap()
    y = nc.dram_tensor("y", [T, C], F32, kind="ExternalOutput").ap()

    with tile.TileContext(nc) as tc, ExitStack() as ctx:
        big = ctx.enter_context(tc.tile_pool(name="big", bufs=1))

        # persistent activations / weights
        qt_sb = big.tile([128, G * T], BF16)  # [d, h*T + t]
        kt_sb = big.tile([128, T], BF16)  # [d, t]
        v_sb = big.tile([128, NKT * HD], BF16)  # [t-part, kt*HD + d]
        wq_sb = big.tile([128, NCC * GW], BF16)  # [c-chunk p, ci*512 + j]
        wk_sb = big.tile([128, NCC * HD], BF16)
        wv_sb = big.tile([128, NCC * HD], BF16)
        cos_sb = big.tile([HD, T], F32)
        sin_sb = big.tile([HD, T], F32)
        wo_sb = big.tile([128, G * C], BF16)  # [j in head-chunk, h*C + c]
        mask_sb = big.tile([128, 128], F32)
        ident_sb = big.tile([128, 128], BF16)
        ones_sb = big.tile([128, 128], F32R)

        # rotating pools
        x_pool = ctx.enter_context(tc.tile_pool(name="xp", bufs=2))
        rp = ctx.enter_context(tc.tile_pool(name="rp", bufs=2))
        pt_pool = ctx.enter_context(tc.tile_pool(name="pt", bufs=6))
        pacc_pool = ctx.enter_context(tc.tile_pool(name="pacc", bufs=4))
        nrm_pool = ctx.enter_context(tc.tile_pool(name="nrm", bufs=2))
        ot_pool = ctx.enter_context(tc.tile_pool(name="ot", bufs=2))
        ysb_pool = ctx.enter_context(tc.tile_pool(name="ysb", bufs=2))

        # ---- upfront DMA issue ----
        # weights + constants on the scalar (ACT) queue: idle until first rope
        for ci in range(NCC):
            nc.scalar.dma_start(
                wq_sb[:, ci * GW : (ci + 1) * GW], wq[ci * 128 : (ci + 1) * 128, :]
            )
            if ci == 0:
                nc.scalar.dma_start(
                    wk_sb[:].rearrange("p (ci j) -> p ci j", ci=NCC),
                    wk.rearrange("(ci p) j -> p ci j", p=128),
                )
                nc.scalar.dma_start(
                    wv_sb[:].rearrange("p (ci j) -> p ci j", ci=NCC),
                    wv.rearrange("(ci p) j -> p ci j", p=128),
                )
        nc.scalar.dma_start(cos_sb[:], cos[:])
        nc.scalar.dma_start(sin_sb[:], sin[:])
        nc.scalar.dma_start(mask_sb[:], masks[:])
        nc.scalar.dma_start(ident_sb[:], ident[:])
        nc.scalar.dma_start(ones_sb[:], onesfull.bitcast(F32R))
        nc.scalar.dma_start(
            wo_sb[:].rearrange("p (h c) -> p h c", h=G),
            wo.rearrange("(h p) c -> p h c", p=128),
        )

        # x t-chunks on the sync queue; one DMA per chunk
        x_tiles = {}

        def issue_x(tci):
            xt = x_pool.tile([128, NCC * TC], BF16, tag="x", name=f"x{tci}")
            ts = slice(tci * TC, (tci + 1) * TC)
            nc.sync.dma_start(
                xt[:].rearrange("p (ci t) -> p ci t", ci=NCC),
                xT[:, ts].rearrange("(ci p) t -> p ci t", p=128),
            )
            x_tiles[tci] = xt

        issue_x(0)
        issue_x(1)

        # ---------------- projection segment for t-chunk tci ----------------
        def proj_segment(tci):
            ts = slice(tci * TC, (tci + 1) * TC)
            with ExitStack() as seg:
                qt_ps_pool = seg.enter_context(
                    tc.tile_pool(name=f"qtps{tci}", bufs=4, space="PSUM")
                )
                kv_ps_pool = seg.enter_context(
                    tc.tile_pool(name=f"kvps{tci}", bufs=2, space="PSUM")
                )
                tp_ps_pool = seg.enter_context(
                    tc.tile_pool(name=f"tpps{tci}", bufs=1, space="PSUM")
                )
                qt_ps = [
                    qt_ps_pool.tile([128, TC], F32, tag="qtps", name=f"qtps{tci}_{j}")
                    for j in range(G)
                ]
                kt_ps = kv_ps_pool.tile([128, TC], F32, tag="kvps", name=f"ktps{tci}")
                vt_ps = kv_ps_pool.tile([128, TC], F32, tag="kvps", name=f"vtps{tci}")
                xt = x_tiles[tci]
                for ci in range(NCC):
                    xs = xt[:, ci * TC : (ci + 1) * TC]
                    st, sp = (ci == 0), (ci == NCC - 1)
                    nc.tensor.matmul(
                        kt_ps[:], wk_sb[:, ci * HD : (ci + 1) * HD], xs, start=st, stop=sp
                    )
                    nc.tensor.matmul(
                        vt_ps[:], wv_sb[:, ci * HD : (ci + 1) * HD], xs, start=st, stop=sp
                    )
                    for j in range(G):
                        nc.tensor.matmul(
                            qt_ps[j][:],
                            wq_sb[:, ci * GW + j * HD : ci * GW + (j + 1) * HD],
                            xs,
                            start=st,
                            stop=sp,
                        )

                # rope: out = q*cos + swap(q)*sin_signed.  K first (S-matmuls
                # need it), then heads; muls/adds split DVE / gpsimd, copies
                # and half-swaps on the scalar queue.
                def rope(ps, out_ap, eng, name):
                    raw = rp.tile([128, TC], F32, tag="raw", name=f"raw_{name}")
                    nc.scalar.copy(raw[:], ps[:])
                    t1 = rp.tile([128, TC], F32, tag="t1", name=f"t1_{name}")
                    eng.tensor_mul(t1[:], raw[:], cos_sb[:, ts])
                    sw = rp.tile([128, TC], F32, tag="sw", name=f"sw_{name}")
                    nc.scalar.dma_start(sw[0:64, :], raw[64:128, :])
                    nc.scalar.dma_start(sw[64:128, :], raw[0:64, :])
                    t2 = rp.tile([128, TC], F32, tag="t2", name=f"t2_{name}")
                    eng.tensor_mul(t2[:], sw[:], sin_sb[:, ts])
                    eng.tensor_add(out_ap, t1[:], t2[:])

                rope(kt_ps, kt_sb[:, ts], nc.vector, f"k{tci}")
                rope(qt_ps[0], qt_sb[:, 0 * T + tci * TC : 0 * T + (tci + 1) * TC],
                     nc.vector, f"q{tci}_0")
                rope(qt_ps[1], qt_sb[:, 1 * T + tci * TC : 1 * T + (tci + 1) * TC],
                     nc.gpsimd, f"q{tci}_1")
                rope(qt_ps[2], qt_sb[:, 2 * T + tci * TC : 2 * T + (tci + 1) * TC],
                     nc.vector, f"q{tci}_2")
                rope(qt_ps[3], qt_sb[:, 3 * T + tci * TC : 3 * T + (tci + 1) * TC],
                     nc.gpsimd, f"q{tci}_3")

                # V: [d, t] psum -> sbuf bf16, then PE-transpose to [t, d]
                vt_f = rp.tile([128, TC], BF16, tag="vtf", name=f"vtf{tci}")
                nc.scalar.copy(vt_f[:], vt_ps[:])
                for s in range(TC // 128):
                    kt_i = tci * (TC // 128) + s
                    tp_ps = tp_ps_pool.tile(
                        [128, 128], BF16, tag="tp", name=f"tp{kt_i}"
                    )
                    nc.tensor.transpose(
                        tp_ps[:], vt_f[:, s * 128 : (s + 1) * 128], ident_sb[:]
                    )
                    nc.scalar.copy(v_sb[:, kt_i * HD : (kt_i + 1) * HD], tp_ps[:])

        # ---------------- attention segment for q-block qb ----------------
        def attn_segment(qb):
            nkt = (qb + 1) * (TC // 128)
            if qb + 2 < NTC:
                issue_x(qb + 2)
            with ExitStack() as seg:
                st_pool = seg.enter_context(
                    tc.tile_pool(name=f"stps{qb}", bufs=4, space="PSUM")
                )
                sps_pool = seg.enter_context(
                    tc.tile_pool(name=f"sps{qb}", bufs=2, space="PSUM")
                )
                ot_ps_pool = seg.enter_context(
                    tc.tile_pool(name=f"otps{qb}", bufs=2, space="PSUM")
                )
                ot_qb = ot_pool.tile([128, G * TC], BF16, tag="ot", name=f"ot{qb}")
                for hg in range(G // 2):
                    ot_ps = [
                        ot_ps_pool.tile(
                            [128, TC], F32, tag="otps", name=f"otps{qb}_{hg}_{hh}"
                        )
                        for hh in range(2)
                    ]
                    pacc = [
                        pacc_pool.tile(
                            [128, TC], F32R, tag="pacc", name=f"pacc{qb}_{hg}_{hh}"
                        )
                        for hh in range(2)
                    ]
                    pts = {}

                    def emit_s(kt):
                        dj = kt - 4 * qb
                        f0 = max(dj, 0) * 128
                        for hh in range(2):
                            h = 2 * hg + hh
                            s_t = st_pool.tile(
                                [128, TC], F32, tag="st", name=f"st{qb}_{kt}_{h}"
                            )
                            nc.tensor.matmul(
                                s_t[:, f0:TC],
                                kt_sb[:, kt * 128 : (kt + 1) * 128],
                                qt_sb[
                                    :, h * T + qb * TC + f0 : h * T + (qb + 1) * TC
                                ],
                                start=True,
                                stop=True,
                            )
                            if dj >= 0:
                                nc.vector.tensor_add(
                                    s_t[:, f0 : f0 + 128],
                                    s_t[:, f0 : f0 + 128],
                                    mask_sb[:],
                                )
                            pt = pt_pool.tile(
                                [128, TC], BF16, tag="pt", name=f"pt{qb}_{kt}_{h}"
                            )
                            nc.scalar.activation(
                                pt[:, f0:TC],
                                s_t[:, f0:TC],
                                mybir.ActivationFunctionType.Exp,
                            )
                            pts[(kt, hh)] = (pt, f0)

                    def emit_acc(kt):
                        st, sp = (kt == 0), (kt == nkt - 1)
                        for hh in range(2):
                            pt, f0 = pts.pop((kt, hh))
                            eng = nc.vector if hh == 0 else nc.gpsimd
                            if kt == 0:
                                eng.tensor_copy(pacc[hh][:], pt[:])
                            else:
                                eng.tensor_add(
                                    pacc[hh][:, f0:TC],
                                    pacc[hh][:, f0:TC].bitcast(F32),
                                    pt[:, f0:TC],
                                )
                            nc.tensor.matmul(
                                ot_ps[hh][:, f0:TC],
                                v_sb[:, kt * HD : (kt + 1) * HD],
                                pt[:, f0:TC],
                                start=st,
                                stop=sp,
                                skip_group_check=True,
                            )

                    emit_s(0)
                    for kt in range(1, nkt):
                        emit_s(kt)
                        emit_acc(kt - 1)
                    emit_acc(nkt - 1)

                    for hh in range(2):
                        h = 2 * hg + hh
                        sb_ps = sps_pool.tile(
                            [128, TC], F32, tag="sps", name=f"sps{qb}_{hg}_{hh}"
                        )
                        nc.tensor.matmul(
                            sb_ps[:],
                            ones_sb[:],
                            pacc[hh][:],
                            start=True,
                            stop=True,
                        )
                        r_f = nrm_pool.tile(
                            [128, TC], F32, tag="rf", name=f"rf{qb}_{h}"
                        )
                        nc.vector.reciprocal_approx_fast(r_f[:], sb_ps[:])
                        nc.vector.tensor_mul(
                            ot_qb[:, h * TC : (h + 1) * TC], ot_ps[hh][:], r_f[:]
                        )

                # out-projection for this q-block (st_pool slots are free now)
                for tl in range(TC // 128):
                    tsub = qb * (TC // 128) + tl
                    ysb = ysb_pool.tile([128, C], F32, tag="ysb", name=f"ysb{tsub}")
                    for cc in range(C // TC):
                        y_ps = st_pool.tile(
                            [128, TC], F32, tag="st", name=f"yps{tsub}_{cc}"
                        )
                        for h in range(G):
                            nc.tensor.matmul(
                                y_ps[:],
                                ot_qb[:, h * TC + tl * 128 : h * TC + (tl + 1) * 128],
                                wo_sb[:, h * C + cc * TC : h * C + (cc + 1) * TC],
                                start=(h == 0),
                                stop=(h == G - 1),
                            )
                        if cc % 2 == 0:
                            nc.vector.tensor_copy(ysb[:, cc * TC : (cc + 1) * TC], y_ps[:])
                        else:
                            nc.scalar.copy(ysb[:, cc * TC : (cc + 1) * TC], y_ps[:])
                    nc.sync.dma_start(
                        y[tsub * 128 : (tsub + 1) * 128, :], ysb[:]
                    )

        for tci in range(NTC):
            proj_segment(tci)
            attn_segment(tci)

    nc.compile()
    return nc


def _rope_tables():
    theta = 1.0 / (10000.0 ** (np.arange(0, HD, 2, dtype=np.float32) / HD))
    freqs = np.arange(T, dtype=np.float32)[:, None] * theta[None, :]  # [T, 64]
    cos = np.concatenate([np.cos(freqs), np.cos(freqs)], axis=-1)  # [T, 128]
    sin = np.concatenate([np.sin(freqs), np.sin(freqs)], axis=-1)
    cosT = np.ascontiguousarray(cos.T).astype(np.float32)  # [128, T]
    sinT = np.ascontiguousarray(sin.T).astype(np.float32)
    sign = np.where(np.arange(HD) < HD // 2, np.float32(-1.0), np.float32(1.0))[:, None]
    sinT_signed = (sinT * sign).astype(np.float32)
    return cosT, sinT_signed


def _masks():
    p = np.arange(128)[:, None]
    f = np.arange(128)[None, :]
    return np.where(p <= f, 0.0, MASK_NEG).astype(np.float32)


def _bf16(a):
    return np.ascontiguousarray(a).astype(ml_dtypes.bfloat16)


def make_in_maps(x, Wq, Wk, Wv, Wo):
    x = np.asarray(x, dtype=np.float32)
    Wq = np.asarray(Wq, dtype=np.float32)
    Wk = np.asarray(Wk, dtype=np.float32)
    Wv = np.asarray(Wv, dtype=np.float32)
    Wo = np.asarray(Wo, dtype=np.float32)

    cosT, sinT = _rope_tables()
    masks = _masks()
    qscale = np.float32(1.0 / np.sqrt(HD))
    ident = np.eye(128, dtype=np.float32)
    onesfull = np.ones((128, 128), dtype=np.float32)

    in_maps = []
    for c in range(N_CORES):
        b, g = divmod(c, N_KV_HEADS)
        in_maps.append(
            {
                "xT": _bf16(x[b].T),
                "wq": _bf16(Wq[:, g * GW : (g + 1) * GW] * qscale),
                "wk": _bf16(Wk[:, g * HD : (g + 1) * HD]),
                "wv": _bf16(Wv[:, g * HD : (g + 1) * HD]),
                "wo": _bf16(Wo[g * GW : (g + 1) * GW, :]),
                "cos": cosT,
                "sin": sinT,
                "masks": masks,
                "ident": _bf16(ident),
                "onesfull": onesfull,
            }
        )
    return in_maps


def kernel(x, Wq, Wk, Wv, Wo):
    if "nc" not in _prog_cache:
        _prog_cache["nc"] = _build_program()
    nc = _prog_cache["nc"]

    in_maps = make_in_maps(x, Wq, Wk, Wv, Wo)
    res = run_bass_kernel_spmd(nc, in_maps, list(range(N_CORES)))
    _prog_cache["last_results"] = res

    out = np.zeros((B, T, C), dtype=np.float32)
    for c in range(N_CORES):
        b = c // N_KV_HEADS
        out[b] += res.results[c]["y"]
    return out
